# revision 9
# baseline (speedup 1.0000x reference)
"""AdaptiveSampler Trainium2 kernel: batch-parallel frame gather across 8 NeuronCores.

Reference semantics: out[b, j*4+g] = x[b, ceil(mu[b,j,g])] (zero frame when the
sampled index falls outside [0, T-1]), with
  mu[b,j,g] = (dt[b,j]*31.5 + 31.5) + (g - 1.5) * ((64/3 - 1)*delta_t[b,j] + 1).

Strategy: pure data parallelism over batch (4 samples/core). The sampled frame
indices are computed host-side (bit-identical to the jax reference, on jax-CPU)
and shipped as a tiny int32 tensor. On-device the kernel is an indirect-DMA
gather (HBM->SBUF) + indirect scatter (SBUF->HBM); out-of-range anchors are
skipped on both sides (descriptor-level skip via bounds_check), so zero frames
come from the pre-zeroed output buffer and cost no HBM traffic. Frames sampled
more than once per core are fetched once and scattered to up to RMAX output
slots via replicated scatter rounds.

Hardware facts baked in (measured on trn2 via neuron-profile):
- indirect-DMA row size is a 16-bit byte field -> frames are split into SUB=4
  subrows of 37632 B.
- partition-sliced indirect DMAs fail at runtime -> every DMA spans all 128
  partitions; the 256 output subrows live in 2 column blocks.
- descriptor -> SDMA-engine mapping is engine(p) = ((p//4) % 8)*2 + p//64;
  OOB slots cost a 4-byte dummy packet. The host assigns subrows to SBUF
  slots so all 16 engines carry equal byte counts.
"""

import numpy as np

import concourse.bass as bass
import concourse.mybir as mybir
from concourse.bass_utils import run_bass_kernel_spmd

B, T, C, H, W = 32, 64, 3, 112, 112
AOT = 4                      # output frames per anchor; 4 anchors
NCORES = 8
BL = B // NCORES             # local batches per core
CHW = C * H * W              # 37632 floats per frame
SUB = 4                      # subrows per frame (row bytes must be < 64 KiB)
SUBLEN = CHW // SUB          # 9408 floats = 37632 B per subrow
NROWS_IN = BL * T * SUB      # 1024 source subrows per core
FRAMES_OUT = BL * AOT * AOT  # 64 output frames per core
NROWS_OUT = FRAMES_OUT * SUB # 256 output subrows per core
NPART = 128
NBLK = NROWS_OUT // NPART    # 2 column blocks in SBUF
RMAX = 3                     # scatter replication rounds (dedup of repeated frames)
OOB = 1 << 30

TRACE = False
RUN_KWARGS = {}
LAST_RESULT = None

_graph_cache = {}

# engine e owns these 8 partitions (2 quads): measured SDMA striping rule
_PART_E = [
    [4 * (e // 2) + 64 * (e % 2) + 32 * (j // 4) + j % 4 for j in range(8)]
    for e in range(16)
]


def _build_graph():
    nc = bass.Bass()
    xz = nc.declare_dram_parameter("xz", [NROWS_IN, SUBLEN], mybir.dt.float32, isOutput=False)
    ncols = NBLK * (1 + RMAX)
    idx = nc.declare_dram_parameter("idx", [NPART, ncols], mybir.dt.int32, isOutput=False)
    out = nc.declare_dram_parameter("out", [NROWS_OUT, SUBLEN], mybir.dt.float32, isOutput=True)

    with (
        nc.sbuf_tensor("buf", [NPART, NBLK * SUBLEN], mybir.dt.float32) as buf,
        nc.sbuf_tensor("idxs", [NPART, ncols], mybir.dt.int32) as idxs,
        nc.semaphore("s_idx") as s_idx,
        nc.semaphore("s_g") as s_g,
        nc.semaphore("s_s") as s_s,
        nc.Block() as block,
    ):
        @block.sync
        def _(sync):
            sync.dma_start(out=idxs[:, :], in_=idx[:, :]).then_inc(s_idx, 16)

        @block.gpsimd
        def _(gpsimd):
            gpsimd.wait_ge(s_idx, 16)
            rb_in = gpsimd.to_reg(NROWS_IN - 1)
            rb_out = gpsimd.to_reg(NROWS_OUT - 1)

            def gather(blk):
                gpsimd.indirect_dma_start(
                    out=buf[:, blk * SUBLEN:(blk + 1) * SUBLEN],
                    out_offset=None,
                    in_=xz[:, :],
                    in_offset=bass.IndirectOffsetOnAxis(ap=idxs[:, blk:blk + 1], axis=0),
                    bounds_check=rb_in,
                    oob_is_err=False,
                ).then_inc(s_g, 16)

            def scatter(blk, r):
                col = NBLK + blk * RMAX + r
                gpsimd.indirect_dma_start(
                    out=out[:, :],
                    out_offset=bass.IndirectOffsetOnAxis(ap=idxs[:, col:col + 1], axis=0),
                    in_=buf[:, blk * SUBLEN:(blk + 1) * SUBLEN],
                    in_offset=None,
                    bounds_check=rb_out,
                    oob_is_err=False,
                ).then_inc(s_s, 16)

            for blk in range(NBLK):
                gather(blk)
            for blk in range(NBLK):
                gpsimd.wait_ge(s_g, 16 * (blk + 1))
                for r in range(RMAX):
                    scatter(blk, r)
            gpsimd.wait_ge(s_s, 16 * NBLK * RMAX)

    return nc


def _get_graph():
    if "nc" not in _graph_cache:
        _graph_cache["nc"] = _build_graph()
    return _graph_cache["nc"]


def _frame_indices(dt, delta_t):
    """ceil(mu) per (b, j, g), bit-identical to the jax reference (on jax-CPU)."""
    import jax
    import jax.numpy as jnp

    with jax.default_device(jax.devices("cpu")[0]):
        dtj = jnp.asarray(np.asarray(dt, dtype=np.float32))
        dlj = jnp.asarray(np.asarray(delta_t, dtype=np.float32))
        anchor_t = (T - 1) / 2.0
        dts = dtj * anchor_t + anchor_t
        deltas = (T / (AOT - 1) - 1.0) * dlj + 1.0
        grid = jnp.arange(AOT, dtype=jnp.float32)
        mu = dts[:, :, None] + (grid[None, None, :] - (AOT - 1) / 2.0) * deltas[:, :, None]
        idxf = np.asarray(jnp.ceil(mu))  # [B, AOT, AOT] float32
    valid = (idxf >= 0) & (idxf <= T - 1)
    t_idx = np.where(valid, idxf, 0).astype(np.int64)
    return t_idx.reshape(B, AOT * AOT), valid.reshape(B, AOT * AOT)


def _core_index_map(t_flat, v_flat, m):
    """Build the [NPART, NBLK*(1+RMAX)] slot table for core m."""
    # unique (local-batch, frame) -> list of output frames q needing it
    occ = {}
    for q in range(FRAMES_OUT):
        bl, f = q // (AOT * AOT), q % (AOT * AOT)
        b = m * BL + bl
        if v_flat[b, f]:
            occ.setdefault((bl, int(t_flat[b, f])), []).append(q)

    # one gather unit per stored frame copy; each serves <= RMAX output frames
    subunits = []  # (src subrow, [dst subrows])
    for (bl, t), qs in occ.items():
        for c in range(0, len(qs), RMAX):
            grp = qs[c:c + RMAX]
            for su in range(SUB):
                subunits.append((SUB * (bl * T + t) + su, [SUB * q + su for q in grp]))

    # greedy balance: weight = descriptors (1 gather + len(dsts) scatters)
    order = sorted(range(len(subunits)), key=lambda i: -len(subunits[i][1]))
    loads = [0] * 16
    used = [0] * 16
    idx_np = np.full((NPART, NBLK * (1 + RMAX)), OOB, np.int32)
    for i in order:
        src, dsts = subunits[i]
        e = min((e for e in range(16) if used[e] < 2 * 8), key=lambda e: loads[e])
        k = used[e]
        used[e] += 1
        loads[e] += 1 + len(dsts)
        blk = k % NBLK
        part = _PART_E[e][k // NBLK]
        idx_np[part, blk] = src
        for r, d in enumerate(dsts):
            idx_np[part, NBLK + blk * RMAX + r] = d
    return idx_np


def kernel(x, dt, delta_t):
    global LAST_RESULT
    x = np.ascontiguousarray(np.asarray(x), dtype=np.float32)
    t_flat, v_flat = _frame_indices(dt, delta_t)

    in_maps = []
    for m in range(NCORES):
        xs = x[m * BL:(m + 1) * BL].reshape(NROWS_IN, SUBLEN)
        in_maps.append({"xz": xs, "idx": _core_index_map(t_flat, v_flat, m)})

    nc = _get_graph()
    LAST_RESULT = run_bass_kernel_spmd(
        nc, in_maps, core_ids=list(range(NCORES)), trace=TRACE, **RUN_KWARGS
    )
    outs = [r["out"].reshape(BL, AOT * AOT, C, H, W) for r in LAST_RESULT.results]
    return np.concatenate(outs, axis=0)


# revision 10
# speedup vs baseline: 1.0423x; 1.0423x over previous
"""AdaptiveSampler Trainium2 kernel: batch-parallel frame gather across 8 NeuronCores.

Reference semantics: out[b, j*4+g] = x[b, ceil(mu[b,j,g])] (zero frame when the
sampled index falls outside [0, T-1]), with
  mu[b,j,g] = (dt[b,j]*31.5 + 31.5) + (g - 1.5) * ((64/3 - 1)*delta_t[b,j] + 1).

Strategy: pure data parallelism over batch (4 samples/core). The sampled frame
indices are computed host-side (bit-identical to the jax reference, on jax-CPU)
and shipped as a tiny int32 tensor. On-device the kernel is an indirect-DMA
gather (HBM->SBUF) + indirect scatter (SBUF->HBM); out-of-range anchors are
skipped on both sides (descriptor-level skip via bounds_check), so zero frames
come from the pre-zeroed output buffer and cost no HBM traffic.

Hardware facts baked in (measured on trn2 via neuron-profile):
- indirect-DMA row size is a 16-bit byte field -> frames are split into SUB=3
  subrows of 50176 B (largest size that fits).
- partition-sliced indirect DMAs fail at runtime -> every DMA spans all 128
  partitions; the output subrows live in 2 column blocks.
- descriptor -> SDMA-engine mapping is engine(p) = ((p//4) % 8)*2 + p//64;
  OOB slots cost only a 4-byte dummy packet. The host assigns subrows to SBUF
  slots so all 16 engines carry equal byte counts.
"""

import numpy as np

import concourse.bass as bass
import concourse.mybir as mybir
from concourse.bass_utils import run_bass_kernel_spmd

B, T, C, H, W = 32, 64, 3, 112, 112
AOT = 4                      # output frames per anchor; 4 anchors
NCORES = 8
BL = B // NCORES             # local batches per core
CHW = C * H * W              # 37632 floats per frame
SUB = 3                      # subrows per frame (row bytes must be <= 65535)
SUBLEN = CHW // SUB          # 12544 floats = 50176 B per subrow
NROWS_IN = BL * T * SUB      # 768 source subrows per core
FRAMES_OUT = BL * AOT * AOT  # 64 output frames per core
NROWS_OUT = FRAMES_OUT * SUB # 192 output subrows per core
NPART = 128
NBLK = 2                     # column blocks in SBUF (256 slots >= 192 subrows)
OOB = 1 << 30

TRACE = False
RUN_KWARGS = {}
LAST_RESULT = None

_graph_cache = {}

# engine e owns these 8 partitions (2 quads): measured SDMA striping rule
_PART_E = [
    [4 * (e // 2) + 64 * (e % 2) + 32 * (j // 4) + j % 4 for j in range(8)]
    for e in range(16)
]


def _build_graph():
    nc = bass.Bass()
    xz = nc.declare_dram_parameter("xz", [NROWS_IN, SUBLEN], mybir.dt.float32, isOutput=False)
    idx = nc.declare_dram_parameter("idx", [NPART, 2 * NBLK], mybir.dt.int32, isOutput=False)
    out = nc.declare_dram_parameter("out", [NROWS_OUT, SUBLEN], mybir.dt.float32, isOutput=True)

    with (
        nc.sbuf_tensor("buf", [NPART, NBLK * SUBLEN], mybir.dt.float32) as buf,
        nc.sbuf_tensor("idxs", [NPART, 2 * NBLK], mybir.dt.int32) as idxs,
        nc.semaphore("s_idx") as s_idx,
        nc.semaphore("s_g") as s_g,
        nc.semaphore("s_s") as s_s,
        nc.Block() as block,
    ):
        @block.sync
        def _(sync):
            sync.dma_start(out=idxs[:, :], in_=idx[:, :]).then_inc(s_idx, 16)

        @block.gpsimd
        def _(gpsimd):
            rb_in = gpsimd.to_reg(NROWS_IN - 1)
            rb_out = gpsimd.to_reg(NROWS_OUT - 1)
            gpsimd.wait_ge(s_idx, 16)

            def gather(blk):
                gpsimd.indirect_dma_start(
                    out=buf[:, blk * SUBLEN:(blk + 1) * SUBLEN],
                    out_offset=None,
                    in_=xz[:, :],
                    in_offset=bass.IndirectOffsetOnAxis(ap=idxs[:, 2 * blk:2 * blk + 1], axis=0),
                    bounds_check=rb_in,
                    oob_is_err=False,
                ).then_inc(s_g, 16)

            def scatter(blk):
                gpsimd.indirect_dma_start(
                    out=out[:, :],
                    out_offset=bass.IndirectOffsetOnAxis(ap=idxs[:, 2 * blk + 1:2 * blk + 2], axis=0),
                    in_=buf[:, blk * SUBLEN:(blk + 1) * SUBLEN],
                    in_offset=None,
                    bounds_check=rb_out,
                    oob_is_err=False,
                ).then_inc(s_s, 16)

            for blk in range(NBLK):
                gather(blk)
            for blk in range(NBLK):
                gpsimd.wait_ge(s_g, 16 * (blk + 1))
                scatter(blk)
            gpsimd.wait_ge(s_s, 16 * NBLK)

    return nc


def _get_graph():
    if "nc" not in _graph_cache:
        _graph_cache["nc"] = _build_graph()
    return _graph_cache["nc"]


def _frame_indices(dt, delta_t):
    """ceil(mu) per (b, j, g), bit-identical to the jax reference (on jax-CPU)."""
    import jax
    import jax.numpy as jnp

    with jax.default_device(jax.devices("cpu")[0]):
        dtj = jnp.asarray(np.asarray(dt, dtype=np.float32))
        dlj = jnp.asarray(np.asarray(delta_t, dtype=np.float32))
        anchor_t = (T - 1) / 2.0
        dts = dtj * anchor_t + anchor_t
        deltas = (T / (AOT - 1) - 1.0) * dlj + 1.0
        grid = jnp.arange(AOT, dtype=jnp.float32)
        mu = dts[:, :, None] + (grid[None, None, :] - (AOT - 1) / 2.0) * deltas[:, :, None]
        idxf = np.asarray(jnp.ceil(mu))  # [B, AOT, AOT] float32
    valid = (idxf >= 0) & (idxf <= T - 1)
    t_idx = np.where(valid, idxf, 0).astype(np.int64)
    return t_idx.reshape(B, AOT * AOT), valid.reshape(B, AOT * AOT)


def kernel(x, dt, delta_t):
    global LAST_RESULT
    x = np.ascontiguousarray(np.asarray(x), dtype=np.float32)
    t_flat, v_flat = _frame_indices(dt, delta_t)

    # valid output subrows, balanced round-robin across the 16 SDMA engines
    # (and across the 2 column-block DMAs within an engine)
    q = np.arange(FRAMES_OUT)
    bl = q // (AOT * AOT)
    f = q % (AOT * AOT)

    in_maps = []
    for m in range(NCORES):
        xs = x[m * BL:(m + 1) * BL].reshape(NROWS_IN, SUBLEN)
        b = m * BL + bl
        okq = v_flat[b, f]
        vq = q[okq]
        # per-frame subrows, contiguous subrows of one frame spread over engines
        dst = (SUB * vq[:, None] + np.arange(SUB)[None, :]).ravel()
        tsrc = t_flat[b, f][okq]
        src = (SUB * (bl[okq] * T + tsrc)[:, None] + np.arange(SUB)[None, :]).ravel()
        n = len(dst)
        i = np.arange(n)
        eng = i % 16
        rank = i // 16                    # slot rank within engine (0..15)
        blk_a = rank % NBLK
        jj = rank // NBLK                 # which of the engine's 8 partitions
        part_a = 4 * (eng // 2) + 64 * (eng % 2) + 32 * (jj // 4) + jj % 4
        idx_np = np.full((NPART, 2 * NBLK), OOB, np.int32)
        idx_np[part_a, 2 * blk_a] = src
        idx_np[part_a, 2 * blk_a + 1] = dst
        in_maps.append({"xz": xs, "idx": idx_np})

    nc = _get_graph()
    LAST_RESULT = run_bass_kernel_spmd(
        nc, in_maps, core_ids=list(range(NCORES)), trace=TRACE, **RUN_KWARGS
    )
    outs = [r["out"].reshape(BL, AOT * AOT, C, H, W) for r in LAST_RESULT.results]
    return np.concatenate(outs, axis=0)


# revision 12
# speedup vs baseline: 1.1452x; 1.0987x over previous
"""AdaptiveSampler Trainium2 kernel: batch-parallel frame gather across 8 NeuronCores.

Reference semantics: out[b, j*4+g] = x[b, ceil(mu[b,j,g])] (zero frame when the
sampled index falls outside [0, T-1]), with
  mu[b,j,g] = (dt[b,j]*31.5 + 31.5) + (g - 1.5) * ((64/3 - 1)*delta_t[b,j] + 1).

Strategy: pure data parallelism over batch (4 samples/core). The sampled frame
indices are computed host-side (bit-identical to the jax reference, on jax-CPU)
and shipped as a tiny int32 tensor. On-device the kernel is an indirect-DMA
gather (HBM->SBUF) + indirect scatter (SBUF->HBM); out-of-range anchors are
skipped on both sides (descriptor-level skip via bounds_check), so zero frames
come from the pre-zeroed output buffer and cost no HBM traffic.

Hardware facts baked in (measured on trn2 via neuron-profile):
- indirect-DMA row size is a 16-bit byte field -> frames are split into SUB=4
  subrows of 37632 B (SUB=3 / 50176 B also fits but adds 176 OOB dummy
  packets per core and measures ~6 us slower).
- partition-sliced indirect DMAs fail at runtime -> every DMA spans all 128
  partitions; the output subrows live in 2 column blocks.
- descriptor -> SDMA-engine mapping is engine(p) = ((p//4) % 8)*2 + p//64;
  OOB slots cost only a 4-byte dummy packet. The host assigns subrows to SBUF
  slots so all 16 engines carry equal byte counts.
"""

import numpy as np

import concourse.bass as bass
import concourse.mybir as mybir
from concourse.bass_utils import run_bass_kernel_spmd

B, T, C, H, W = 32, 64, 3, 112, 112
AOT = 4                      # output frames per anchor; 4 anchors
NCORES = 8
BL = B // NCORES             # local batches per core
CHW = C * H * W              # 37632 floats per frame
SUB = 4                      # subrows per frame (row bytes must be <= 65535)
SUBLEN = CHW // SUB          # 9408 floats = 37632 B per subrow
NROWS_IN = BL * T * SUB      # 1024 source subrows per core
FRAMES_OUT = BL * AOT * AOT  # 64 output frames per core
NROWS_OUT = FRAMES_OUT * SUB # 256 output subrows per core
NPART = 128
NBLK = 2                     # column blocks in SBUF (256 slots = 256 subrows)
OOB = 1 << 30

TRACE = False
RUN_KWARGS = {}
LAST_RESULT = None

_graph_cache = {}

# engine e owns these 8 partitions (2 quads): measured SDMA striping rule
_PART_E = [
    [4 * (e // 2) + 64 * (e % 2) + 32 * (j // 4) + j % 4 for j in range(8)]
    for e in range(16)
]


def _build_graph():
    nc = bass.Bass()
    xz = nc.declare_dram_parameter("xz", [NROWS_IN, SUBLEN], mybir.dt.float32, isOutput=False)
    idx = nc.declare_dram_parameter("idx", [NPART, 2 * NBLK], mybir.dt.int32, isOutput=False)
    out = nc.declare_dram_parameter("out", [NROWS_OUT, SUBLEN], mybir.dt.float32, isOutput=True)

    with (
        nc.sbuf_tensor("buf", [NPART, NBLK * SUBLEN], mybir.dt.float32) as buf,
        nc.sbuf_tensor("idxs", [NPART, 2 * NBLK], mybir.dt.int32) as idxs,
        nc.semaphore("s_idx") as s_idx,
        nc.semaphore("s_g") as s_g,
        nc.semaphore("s_s") as s_s,
        nc.Block() as block,
    ):
        @block.sync
        def _(sync):
            sync.dma_start(out=idxs[:, :], in_=idx[:, :]).then_inc(s_idx, 16)

        @block.gpsimd
        def _(gpsimd):
            rb_in = gpsimd.to_reg(NROWS_IN - 1)
            rb_out = gpsimd.to_reg(NROWS_OUT - 1)
            gpsimd.wait_ge(s_idx, 16)

            def gather(blk):
                gpsimd.indirect_dma_start(
                    out=buf[:, blk * SUBLEN:(blk + 1) * SUBLEN],
                    out_offset=None,
                    in_=xz[:, :],
                    in_offset=bass.IndirectOffsetOnAxis(ap=idxs[:, 2 * blk:2 * blk + 1], axis=0),
                    bounds_check=rb_in,
                    oob_is_err=False,
                ).then_inc(s_g, 16)

            def scatter(blk):
                gpsimd.indirect_dma_start(
                    out=out[:, :],
                    out_offset=bass.IndirectOffsetOnAxis(ap=idxs[:, 2 * blk + 1:2 * blk + 2], axis=0),
                    in_=buf[:, blk * SUBLEN:(blk + 1) * SUBLEN],
                    in_offset=None,
                    bounds_check=rb_out,
                    oob_is_err=False,
                ).then_inc(s_s, 16)

            for blk in range(NBLK):
                gather(blk)
            for blk in range(NBLK):
                gpsimd.wait_ge(s_g, 16 * (blk + 1))
                scatter(blk)
            gpsimd.wait_ge(s_s, 16 * NBLK)

    return nc


def _get_graph():
    if "nc" not in _graph_cache:
        _graph_cache["nc"] = _build_graph()
    return _graph_cache["nc"]


def _frame_indices(dt, delta_t):
    """ceil(mu) per (b, j, g), bit-identical to the jax reference (on jax-CPU)."""
    import jax
    import jax.numpy as jnp

    with jax.default_device(jax.devices("cpu")[0]):
        dtj = jnp.asarray(np.asarray(dt, dtype=np.float32))
        dlj = jnp.asarray(np.asarray(delta_t, dtype=np.float32))
        anchor_t = (T - 1) / 2.0
        dts = dtj * anchor_t + anchor_t
        deltas = (T / (AOT - 1) - 1.0) * dlj + 1.0
        grid = jnp.arange(AOT, dtype=jnp.float32)
        mu = dts[:, :, None] + (grid[None, None, :] - (AOT - 1) / 2.0) * deltas[:, :, None]
        idxf = np.asarray(jnp.ceil(mu))  # [B, AOT, AOT] float32
    valid = (idxf >= 0) & (idxf <= T - 1)
    t_idx = np.where(valid, idxf, 0).astype(np.int64)
    return t_idx.reshape(B, AOT * AOT), valid.reshape(B, AOT * AOT)


def kernel(x, dt, delta_t):
    global LAST_RESULT
    x = np.ascontiguousarray(np.asarray(x), dtype=np.float32)
    t_flat, v_flat = _frame_indices(dt, delta_t)

    # valid output subrows, balanced round-robin across the 16 SDMA engines
    # (and across the 2 column-block DMAs within an engine)
    q = np.arange(FRAMES_OUT)
    bl = q // (AOT * AOT)
    f = q % (AOT * AOT)

    in_maps = []
    for m in range(NCORES):
        xs = x[m * BL:(m + 1) * BL].reshape(NROWS_IN, SUBLEN)
        b = m * BL + bl
        okq = v_flat[b, f]
        vq = q[okq]
        # per-frame subrows, contiguous subrows of one frame spread over engines
        dst = (SUB * vq[:, None] + np.arange(SUB)[None, :]).ravel()
        tsrc = t_flat[b, f][okq]
        src = (SUB * (bl[okq] * T + tsrc)[:, None] + np.arange(SUB)[None, :]).ravel()
        n = len(dst)
        i = np.arange(n)
        eng = i % 16
        rank = i // 16                    # slot rank within engine (0..15)
        blk_a = rank % NBLK
        jj = rank // NBLK                 # which of the engine's 8 partitions
        part_a = 4 * (eng // 2) + 64 * (eng % 2) + 32 * (jj // 4) + jj % 4
        idx_np = np.full((NPART, 2 * NBLK), OOB, np.int32)
        idx_np[part_a, 2 * blk_a] = src
        idx_np[part_a, 2 * blk_a + 1] = dst
        in_maps.append({"xz": xs, "idx": idx_np})

    nc = _get_graph()
    LAST_RESULT = run_bass_kernel_spmd(
        nc, in_maps, core_ids=list(range(NCORES)), trace=TRACE, **RUN_KWARGS
    )
    outs = [r["out"].reshape(BL, AOT * AOT, C, H, W) for r in LAST_RESULT.results]
    return np.concatenate(outs, axis=0)


# revision 18
# speedup vs baseline: 1.1781x; 1.0287x over previous
"""AdaptiveSampler Trainium2 kernel: batch-parallel frame gather across 8 NeuronCores.

Reference semantics: out[b, j*4+g] = x[b, ceil(mu[b,j,g])] (zero frame when the
sampled index falls outside [0, T-1]), with
  mu[b,j,g] = (dt[b,j]*31.5 + 31.5) + (g - 1.5) * ((64/3 - 1)*delta_t[b,j] + 1).

Strategy: pure data parallelism over batch (4 samples/core). The sampled frame
indices are computed host-side (bit-identical to the jax reference, on jax-CPU)
and shipped as a tiny int32 tensor. On-device the kernel is an indirect-DMA
gather (HBM->SBUF) + indirect scatter (SBUF->HBM); out-of-range anchors are
skipped on both sides (descriptor-level skip via bounds_check), so zero frames
come from the pre-zeroed output buffer and cost no HBM traffic.

Hardware facts baked in (measured on trn2 via neuron-profile):
- indirect-DMA row size is a 16-bit byte field -> frames are split into SUB=4
  subrows of 37632 B (SUB=3 / 50176 B also fits but adds 176 OOB dummy
  packets per core and measures ~6 us slower).
- partition-sliced indirect DMAs fail at runtime -> every DMA spans all 128
  partitions; the output subrows live in 2 column blocks.
- descriptor -> SDMA-engine mapping is engine(p) = ((p//4) % 8)*2 + p//64;
  OOB slots cost only a 4-byte dummy packet. The host assigns subrows to SBUF
  slots so all 16 engines carry equal byte counts.
"""

import numpy as np

import concourse.bass as bass
import concourse.mybir as mybir
from concourse.bass_utils import run_bass_kernel_spmd

B, T, C, H, W = 32, 64, 3, 112, 112
AOT = 4                      # output frames per anchor; 4 anchors
NCORES = 8
BL = B // NCORES             # local batches per core
CHW = C * H * W              # 37632 floats per frame
SUB = 4                      # subrows per frame (row bytes must be <= 65535)
SUBLEN = CHW // SUB          # 9408 floats = 37632 B per subrow
NROWS_IN = BL * T * SUB      # 1024 source subrows per core
FRAMES_OUT = BL * AOT * AOT  # 64 output frames per core
NROWS_OUT = FRAMES_OUT * SUB # 256 output subrows per core
NPART = 128
NBLK = 2                     # column blocks in SBUF (256 slots = 256 subrows)
OOB = 1 << 30

TRACE = False
RUN_KWARGS = {}
LAST_RESULT = None

_graph_cache = {}

# engine e owns these 8 partitions (2 quads): measured SDMA striping rule
_PART_E = [
    [4 * (e // 2) + 64 * (e % 2) + 32 * (j // 4) + j % 4 for j in range(8)]
    for e in range(16)
]


def _build_graph():
    nc = bass.Bass()
    xz = nc.declare_dram_parameter("xz", [NROWS_IN, SUBLEN], mybir.dt.float32, isOutput=False)
    idx = nc.declare_dram_parameter("idx", [NPART, 2 * NBLK], mybir.dt.int32, isOutput=False)
    out = nc.declare_dram_parameter("out", [NROWS_OUT, SUBLEN], mybir.dt.float32, isOutput=True)

    with (
        nc.sbuf_tensor("buf", [NPART, NBLK * SUBLEN], mybir.dt.float32) as buf,
        nc.sbuf_tensor("idxs", [NPART, 2 * NBLK], mybir.dt.int32) as idxs,
        nc.semaphore("s_idx") as s_idx,
        nc.semaphore("s_g") as s_g,
        nc.semaphore("s_s") as s_s,
        nc.Block() as block,
    ):
        @block.sync
        def _(sync):
            sync.dma_start(out=idxs[:, :], in_=idx[:, :]).then_inc(s_idx, 16)

        @block.gpsimd
        def _(gpsimd):
            rb_in = gpsimd.to_reg(NROWS_IN - 1)
            rb_out = gpsimd.to_reg(NROWS_OUT - 1)
            first = [True]

            def fuse_wait(ins):
                if first[0]:
                    ins._wait_ge(s_idx, 16)
                    first[0] = False
                return ins

            def gather(blk):
                fuse_wait(gpsimd.indirect_dma_start(
                    out=buf[:, blk * SUBLEN:(blk + 1) * SUBLEN],
                    out_offset=None,
                    in_=xz[:, :],
                    in_offset=bass.IndirectOffsetOnAxis(ap=idxs[:, 2 * blk:2 * blk + 1], axis=0),
                    bounds_check=rb_in,
                    oob_is_err=False,
                )).then_inc(s_g, 16)

            def scatter(blk):
                gpsimd.indirect_dma_start(
                    out=out[:, :],
                    out_offset=bass.IndirectOffsetOnAxis(ap=idxs[:, 2 * blk + 1:2 * blk + 2], axis=0),
                    in_=buf[:, blk * SUBLEN:(blk + 1) * SUBLEN],
                    in_offset=None,
                    bounds_check=rb_out,
                    oob_is_err=False,
                )._wait_ge(s_g, 16 * (blk + 1)).then_inc(s_s, 16)

            for blk in range(NBLK):
                gather(blk)
            for blk in range(NBLK):
                scatter(blk)
            gpsimd.wait_ge(s_s, 16 * NBLK)

    return nc


def _get_graph():
    if "nc" not in _graph_cache:
        _graph_cache["nc"] = _build_graph()
    return _graph_cache["nc"]


def _frame_indices(dt, delta_t):
    """ceil(mu) per (b, j, g), bit-identical to the jax reference (on jax-CPU)."""
    import jax
    import jax.numpy as jnp

    with jax.default_device(jax.devices("cpu")[0]):
        dtj = jnp.asarray(np.asarray(dt, dtype=np.float32))
        dlj = jnp.asarray(np.asarray(delta_t, dtype=np.float32))
        anchor_t = (T - 1) / 2.0
        dts = dtj * anchor_t + anchor_t
        deltas = (T / (AOT - 1) - 1.0) * dlj + 1.0
        grid = jnp.arange(AOT, dtype=jnp.float32)
        mu = dts[:, :, None] + (grid[None, None, :] - (AOT - 1) / 2.0) * deltas[:, :, None]
        idxf = np.asarray(jnp.ceil(mu))  # [B, AOT, AOT] float32
    valid = (idxf >= 0) & (idxf <= T - 1)
    t_idx = np.where(valid, idxf, 0).astype(np.int64)
    return t_idx.reshape(B, AOT * AOT), valid.reshape(B, AOT * AOT)


def kernel(x, dt, delta_t):
    global LAST_RESULT
    x = np.ascontiguousarray(np.asarray(x), dtype=np.float32)
    t_flat, v_flat = _frame_indices(dt, delta_t)

    # valid output subrows, balanced round-robin across the 16 SDMA engines
    # (and across the 2 column-block DMAs within an engine)
    q = np.arange(FRAMES_OUT)
    bl = q // (AOT * AOT)
    f = q % (AOT * AOT)

    in_maps = []
    for m in range(NCORES):
        xs = x[m * BL:(m + 1) * BL].reshape(NROWS_IN, SUBLEN)
        b = m * BL + bl
        okq = v_flat[b, f]
        vq = q[okq]
        # per-frame subrows, contiguous subrows of one frame spread over engines
        dst = (SUB * vq[:, None] + np.arange(SUB)[None, :]).ravel()
        tsrc = t_flat[b, f][okq]
        src = (SUB * (bl[okq] * T + tsrc)[:, None] + np.arange(SUB)[None, :]).ravel()
        n = len(dst)
        i = np.arange(n)
        eng = i % 16
        rank = i // 16                    # slot rank within engine (0..15)
        blk_a = rank % NBLK
        jj = rank // NBLK                 # which of the engine's 8 partitions
        part_a = 4 * (eng // 2) + 64 * (eng % 2) + 32 * (jj // 4) + jj % 4
        idx_np = np.full((NPART, 2 * NBLK), OOB, np.int32)
        idx_np[part_a, 2 * blk_a] = src
        idx_np[part_a, 2 * blk_a + 1] = dst
        in_maps.append({"xz": xs, "idx": idx_np})

    nc = _get_graph()
    LAST_RESULT = run_bass_kernel_spmd(
        nc, in_maps, core_ids=list(range(NCORES)), trace=TRACE, **RUN_KWARGS
    )
    outs = [r["out"].reshape(BL, AOT * AOT, C, H, W) for r in LAST_RESULT.results]
    return np.concatenate(outs, axis=0)
